# revision 16
# baseline (speedup 1.0000x reference)
"""Trainium2 Bass kernel for EpsilonNetGM (forward-diffused GMM score network).

Math (per row x of shape [D]):
    m'_k    = sqrt(acp) * means_k
    logit_k = (x . m'_k)/sigma2 + [log w_k - 0.5*||m'_k||^2/sigma2]
    resp    = softmax_k(logit)
    out     = c * (x - resp @ m'),   c = 1/sqrt(sigma2),  sigma2 = 1 - acp

Data-parallel over 8 NeuronCores: x/out sharded on the batch axis.

v3 — single-precision bf16 pipeline (tolerance is 2e-2; sim rel err 2.3e-3):
 - Host uploads x TWICE: transposed bf16 (for mm1's moving operand — no
   DMA/PE transposes on device) and c*x in f16 natural layout (for the
   final add). The transposed copy's columns are permuted so that
   n = 4p + g within each 512-row superblock: mm2 stationary slices stay
   contiguous AND the xc/out DMAs get 1KB-contiguous 4-row runs.
 - Per 512-row superblock: ONE 512-col mm1 (stationary ms [128,25]),
   one exp ACT (bias = logw_adj per-partition), FOUR 129-col mm2s
   (stationary = E^T 128-col slice, moving = [-m' | 1] so the softmax
   denominator lands in an extra PSUM column), four STTs
   out = V*(c/s) + c*x, f16 output store.
 - mm1 of superblock s+1 is issued before mm2 of superblock s so the PE
   never stalls on the exp latency.
"""

import os
import sys

for _p in ("/opt/trn_rl_repo", "/root/.axon_site/_ro/trn_rl_repo"):
    if os.path.isdir(_p) and _p not in sys.path:
        sys.path.insert(0, _p)

import numpy as np
import ml_dtypes
from contextlib import ExitStack

import concourse.bass as bass
import concourse.bacc as bacc
import concourse.tile as tile
from concourse import mybir
from concourse.bass_utils import run_bass_kernel_spmd

N_CORES = 8
N, K, D = 32768, 25, 128
N_PER = N // N_CORES          # 4096 rows per core
SB = 512                      # rows per super-block
NSB = N_PER // SB             # 8 super-blocks per core

F32 = mybir.dt.float32
F16 = mybir.dt.float16
BF16 = mybir.dt.bfloat16
AF = mybir.ActivationFunctionType
OP = mybir.AluOpType


def build_program(c_scale: float):
    nc = bacc.Bacc("TRN2", debug=False)

    xt_d = nc.dram_tensor("xt", [D, N_PER], BF16, kind="ExternalInput").ap()
    xc_d = nc.dram_tensor("xc", [N_PER, D], F16, kind="ExternalInput").ap()
    ms_d = nc.dram_tensor("ms", [D, K], BF16, kind="ExternalInput").ap()
    lw_d = nc.dram_tensor("lw", [K, 1], F32, kind="ExternalInput").ap()
    nma_d = nc.dram_tensor("nma", [K, D + 1], BF16, kind="ExternalInput").ap()
    out_d = nc.dram_tensor("out", [N_PER, D], F16, kind="ExternalOutput").ap()

    inv_c = float(1.0 / c_scale)

    NP = NSB // 2  # pairs of super-blocks (DMA granularity)

    with tile.TileContext(nc) as tc, ExitStack() as ctx:
        consts = ctx.enter_context(tc.tile_pool(name="consts", bufs=1))
        xt_p = ctx.enter_context(tc.tile_pool(name="xt", bufs=4))
        xc_p = ctx.enter_context(tc.tile_pool(name="xc", bufs=4))
        eta_p = ctx.enter_context(tc.tile_pool(name="eta", bufs=4))
        small_p = ctx.enter_context(tc.tile_pool(name="small", bufs=4))
        out_p = ctx.enter_context(tc.tile_pool(name="outp", bufs=3))
        ps_st = ctx.enter_context(tc.tile_pool(name="ps_st", bufs=2, space="PSUM"))
        ps_v = ctx.enter_context(tc.tile_pool(name="ps_v", bufs=3, space="PSUM"))

        etas, xts, xcs, o2s = {}, {}, {}, {}

        def dma_in(p, split=False):
            n0 = p * 2 * SB
            # x^T slice (column-permuted: col 128g+q holds row 4q+g per SB)
            xt = xt_p.tile([128, 2 * SB], BF16, name="xt")
            if split:  # pair 0: half-loads so mm1(0) starts sooner
                nc.sync.dma_start(xt[:, :SB], xt_d[:, n0:n0 + SB])
                nc.sync.dma_start(xt[:, SB:], xt_d[:, n0 + SB:n0 + 2 * SB])
            else:
                # alternate queues -> two concurrent input streams
                eng = nc.scalar if p % 2 else nc.sync
                eng.dma_start(xt, xt_d[:, n0:n0 + 2 * SB])
            xts[p] = xt
            # c*x in f16, layout [q, (h g d)] <- rows n0 + 512h + 4q + g
            xc = xc_p.tile([128, 2 * SB], F16, name="xc")
            nc.gpsimd.dma_start(
                xc.rearrange("q (h g d) -> q h g d", g=4, d=D),
                xc_d[n0:n0 + 2 * SB, :].rearrange("(h q g) d -> q h g d", h=2, g=4),
            )
            xcs[p] = xc

        def head(s):
            # S^T[k, j] = x_j . m'_k / sigma2 ; E^T = exp(S^T + logw_adj)
            p, h = divmod(s, 2)
            xt = xts[p]
            pst = ps_st.tile([K, SB], F32, name="pst")
            nc.tensor.matmul(pst, lhsT=ms, rhs=xt[:, SB * h:SB * (h + 1)],
                             start=True, stop=True)
            eta = eta_p.tile([K, SB], BF16, name="eta")
            nc.scalar.activation(eta, pst, AF.Exp, bias=lw[:, 0:1], scale=1.0)
            etas[s] = eta

        def tail(s):
            p, h = divmod(s, 2)
            eta = etas.pop(s)
            xc = xcs[p]
            if h == 0:
                o2s[p] = out_p.tile([128, 2 * SB], F16, name="o2")
            o2 = o2s[p]
            # V_g = E_g @ [-c*m' | 1]; col 128 of each 129-group = s/c.
            # One 2-bank PSUM tile [q, i, 512]: group g at (i=g//2, 129*(g%2)).
            pv = ps_v.tile([128, 2, SB], F32, name="pv")
            for g in range(4):
                i, j = divmod(g, 2)
                nc.tensor.matmul(
                    pv[:, i:i + 1, (D + 1) * j:(D + 1) * j + D + 1],
                    lhsT=eta[:, 128 * g:128 * (g + 1)],
                    rhs=nma, start=True, stop=True,
                )

            # rc = c/s for all 4 groups in one reciprocal
            rc4 = small_p.tile([128, 4], F32, name="rc4")
            nc.vector.reciprocal(
                rc4.rearrange("q (i j w) -> q i j w", j=2, w=1),
                pv[:, :, :2 * (D + 1)].rearrange(
                    "q i (j y) -> q i j y", y=D + 1)[:, :, :, D:D + 1],
            )
            # out_g = V_g * (c/s) + c*x_g
            for g in range(4):
                i, j = divmod(g, 2)
                o_lo = SB * h + 128 * g
                nc.vector.scalar_tensor_tensor(
                    out=o2[:, o_lo:o_lo + 128],
                    in0=pv[:, i:i + 1, (D + 1) * j:(D + 1) * j + D],
                    scalar=rc4[:, g:g + 1],
                    in1=xc[:, o_lo:o_lo + 128],
                    op0=OP.mult,
                    op1=OP.add,
                )

        def dma_out(p, eng=None, split=False):
            n0 = p * 2 * SB
            o2 = o2s.pop(p)
            src = o2.rearrange("q (h g d) -> q h g d", g=4, d=D)
            dst = out_d[n0:n0 + 2 * SB, :].rearrange(
                "(h q g) d -> q h g d", h=2, g=4)
            if split:  # last pair: halves on separate queues, right away
                nc.sync.dma_start(dst[:, 0:1], src[:, 0:1])
                nc.gpsimd.dma_start(dst[:, 1:2], src[:, 1:2])
            else:
                eng.dma_start(dst, src)

        # consts go on the Scalar queue (idle at start) so the Sync queue
        # issues the first xt load immediately.
        ms = consts.tile([D, K], BF16, name="ms")
        nc.scalar.dma_start(ms, ms_d)
        lw = consts.tile([K, 1], F32, name="lw")
        nc.scalar.dma_start(lw, lw_d)
        nma = consts.tile([K, D + 1], BF16, name="nma")
        nc.scalar.dma_start(nma, nma_d)

        # all input loads issued up front: inputs must never queue behind
        # output stores on the same DMA queue
        for p in range(NP):
            dma_in(p, split=(p == 0))

        for p in range(NP):
            head(2 * p)
            head(2 * p + 1)
            if p:
                tail(2 * p - 1)
                dma_out(p - 1, eng=nc.sync if p % 2 else nc.gpsimd)
            tail(2 * p)
            if p == NP - 1:
                tail(2 * p + 1)
                dma_out(p, split=True)

    nc.compile()
    return nc


def _host_constants(means, weights, alphas_cumprod, t):
    acp = float(np.asarray(alphas_cumprod, dtype=np.float64)[int(t)])
    sigma2 = 1.0 - acp
    c = 1.0 / np.sqrt(sigma2)
    mprime = np.sqrt(acp) * np.asarray(means, dtype=np.float64)      # [K, D]

    ms = (mprime / sigma2).T.astype(np.float32)                      # [D, K]
    ms = ms.astype(ml_dtypes.bfloat16)

    # Scales folded into constants: E' = E/c (via -ln c in the bias) and
    # nma = [-c*m' | 1], so the ones column accumulates s/c and
    # out = (E'@nma) * (c/s) + c*x = -(E@m')*c/s + c*x directly.
    logw = np.log(np.asarray(weights, dtype=np.float64))
    lw = (logw - 0.5 * np.sum(mprime * mprime, axis=1) / sigma2 - np.log(c))
    lw = lw.astype(np.float32).reshape(K, 1).copy()

    nma = np.zeros((K, D + 1), dtype=np.float32)
    nma[:, :D] = (-c * mprime).astype(np.float32)
    nma[:, D] = 1.0
    nma = nma.astype(ml_dtypes.bfloat16)

    return float(c), ms, lw, nma


def _host_split_x(x, c):
    # Transposed bf16 copy, columns permuted so col (512s + 128g + p)
    # holds row (512s + 4p + g): mm2 stationary slices contiguous.
    v = x.reshape(-1, 128, 4, D)                        # [nsb_t, p, g, d]
    xt = np.ascontiguousarray(
        v.transpose(3, 0, 2, 1).reshape(D, -1).astype(ml_dtypes.bfloat16)
    )
    xc = (np.float32(c) * x).astype(np.float16)
    return xt, xc


def _build(inputs):
    x = np.ascontiguousarray(np.asarray(inputs["x"], dtype=np.float32))
    assert x.shape == (N, D), x.shape
    c, ms, lw, nma = _host_constants(
        inputs["means"], inputs["weights"], inputs["alphas_cumprod"], inputs["t"]
    )

    nc = build_program(c)
    in_maps = []
    for i in range(N_CORES):
        sl = slice(i * N_PER, (i + 1) * N_PER)
        xt, xc = _host_split_x(x[sl], c)
        in_maps.append({
            "xt": xt, "xc": xc,
            "ms": ms, "lw": lw, "nma": nma,
        })
    return nc, in_maps


def kernel(x, means, weights, alphas_cumprod, t):
    nc, in_maps = _build({
        "x": x, "means": means, "weights": weights,
        "alphas_cumprod": alphas_cumprod, "t": t,
    })
    res = run_bass_kernel_spmd(nc, in_maps, list(range(N_CORES)))
    out = np.concatenate([res.results[i]["out"] for i in range(N_CORES)], axis=0)
    return out.astype(np.float32, copy=False)


if __name__ == "__main__":
    rng = np.random.default_rng(0)
    x = rng.standard_normal((N, D), dtype=np.float32)
    means = 2.0 * rng.standard_normal((K, D)).astype(np.float32)
    w = rng.uniform(0.1, 1.0, K).astype(np.float32)
    weights = w / w.sum()
    betas = np.linspace(1e-4, 0.02, 1000, dtype=np.float32)
    acp = np.cumprod(1.0 - betas).astype(np.float32)
    out = kernel(x, means, weights, acp, 500)
    print("out", out.shape, out.dtype, out[:2, :4])


# revision 24
# speedup vs baseline: 1.0235x; 1.0235x over previous
"""Trainium2 Bass kernel for EpsilonNetGM (forward-diffused GMM score network).

Math (per row x of shape [D]):
    m'_k    = sqrt(acp) * means_k
    logit_k = (x . m'_k)/sigma2 + [log w_k - 0.5*||m'_k||^2/sigma2]
    resp    = softmax_k(logit)
    out     = c * (x - resp @ m'),   c = 1/sqrt(sigma2),  sigma2 = 1 - acp

Data-parallel over 8 NeuronCores: x/out sharded on the batch axis.

v3 — single-precision bf16 pipeline (tolerance is 2e-2; sim rel err 2.3e-3):
 - Host uploads x TWICE: transposed bf16 (for mm1's moving operand — no
   DMA/PE transposes on device) and c*x in f16 natural layout (for the
   final add). The transposed copy's columns are permuted so that
   n = 4p + g within each 512-row superblock: mm2 stationary slices stay
   contiguous AND the xc/out DMAs get 1KB-contiguous 4-row runs.
 - Per 512-row superblock: ONE 512-col mm1 (stationary ms [128,25]),
   one exp ACT (bias = logw_adj per-partition), FOUR 129-col mm2s
   (stationary = E^T 128-col slice, moving = [-m' | 1] so the softmax
   denominator lands in an extra PSUM column), four STTs
   out = V*(c/s) + c*x, f16 output store.
 - mm1 of superblock s+1 is issued before mm2 of superblock s so the PE
   never stalls on the exp latency.
"""

import os
import sys

for _p in ("/opt/trn_rl_repo", "/root/.axon_site/_ro/trn_rl_repo"):
    if os.path.isdir(_p) and _p not in sys.path:
        sys.path.insert(0, _p)

import numpy as np
import ml_dtypes
from contextlib import ExitStack

import concourse.bass as bass
import concourse.bacc as bacc
import concourse.tile as tile
from concourse import mybir
from concourse.bass_utils import run_bass_kernel_spmd

N_CORES = 8
N, K, D = 32768, 25, 128
N_PER = N // N_CORES          # 4096 rows per core
SB = 512                      # rows per super-block
NSB = N_PER // SB             # 8 super-blocks per core

F32 = mybir.dt.float32
F16 = mybir.dt.float16
BF16 = mybir.dt.bfloat16
AF = mybir.ActivationFunctionType
OP = mybir.AluOpType


def build_program(c_scale: float):
    nc = bacc.Bacc("TRN2", debug=False)

    xt_d = nc.dram_tensor("xt", [D, N_PER], BF16, kind="ExternalInput").ap()
    xc_d = nc.dram_tensor("xc", [N_PER, D], F16, kind="ExternalInput").ap()
    ms_d = nc.dram_tensor("ms", [D, K], BF16, kind="ExternalInput").ap()
    lw_d = nc.dram_tensor("lw", [K, 1], F32, kind="ExternalInput").ap()
    nma_d = nc.dram_tensor("nma", [K, D + 1], BF16, kind="ExternalInput").ap()
    out_d = nc.dram_tensor("out", [N_PER, D], F16, kind="ExternalOutput").ap()

    inv_c = float(1.0 / c_scale)

    NP = NSB // 2  # pairs of super-blocks (DMA granularity)

    with tile.TileContext(nc) as tc, ExitStack() as ctx:
        consts = ctx.enter_context(tc.tile_pool(name="consts", bufs=1))
        xt_p = ctx.enter_context(tc.tile_pool(name="xt", bufs=4))
        xc_p = ctx.enter_context(tc.tile_pool(name="xc", bufs=4))
        eta_p = ctx.enter_context(tc.tile_pool(name="eta", bufs=4))
        small_p = ctx.enter_context(tc.tile_pool(name="small", bufs=4))
        out_p = ctx.enter_context(tc.tile_pool(name="outp", bufs=3))
        ps_st = ctx.enter_context(tc.tile_pool(name="ps_st", bufs=2, space="PSUM"))
        ps_v = ctx.enter_context(tc.tile_pool(name="ps_v", bufs=3, space="PSUM"))

        etas, xts, xcs, o2s = {}, {}, {}, {}

        def dma_in(p, split=False):
            n0 = p * 2 * SB
            # x^T slice (column-permuted: col 128g+q holds row 4q+g per SB)
            xt = xt_p.tile([128, 2 * SB], BF16, name="xt")
            if split:  # pair 0: half-loads so mm1(0) starts sooner
                nc.sync.dma_start(xt[:, :SB], xt_d[:, n0:n0 + SB])
                nc.sync.dma_start(xt[:, SB:], xt_d[:, n0 + SB:n0 + 2 * SB])
            else:
                # alternate queues -> two concurrent input streams
                eng = nc.scalar if p % 2 else nc.sync
                eng.dma_start(xt, xt_d[:, n0:n0 + 2 * SB])
            xts[p] = xt
            # c*x in f16, layout [q, (g d)] <- row n0 + 8q + g: each
            # partition holds 8 consecutive rows = one 2KB contiguous run
            xc = xc_p.tile([128, 2 * SB], F16, name="xc")
            nc.gpsimd.dma_start(
                xc.rearrange("q (g d) -> q g d", d=D),
                xc_d[n0:n0 + 2 * SB, :].rearrange("(q g) d -> q g d", g=8),
            )
            xcs[p] = xc

        def head(s):
            # S^T[k, j] = x_j . m'_k / sigma2 ; E^T = exp(S^T + logw_adj)
            p, h = divmod(s, 2)
            xt = xts[p]
            pst = ps_st.tile([K, SB], F32, name="pst")
            nc.tensor.matmul(pst, lhsT=ms, rhs=xt[:, SB * h:SB * (h + 1)],
                             start=True, stop=True)
            eta = eta_p.tile([K, SB], BF16, name="eta")
            nc.scalar.activation(eta, pst, AF.Exp, bias=lw[:, 0:1], scale=1.0)
            etas[s] = eta

        def tail(s):
            p, h = divmod(s, 2)
            eta = etas.pop(s)
            xc = xcs[p]
            if h == 0:
                o2s[p] = out_p.tile([128, 2 * SB], F16, name="o2")
            o2 = o2s[p]
            # V_g = E_g @ [-c*m' | 1]; col 128 of each 129-group = s/c.
            # One 2-bank PSUM tile [q, i, 512]: group g at (i=g//2, 129*(g%2)).
            pv = ps_v.tile([128, 2, SB], F32, name="pv")
            for g in range(4):
                i, j = divmod(g, 2)
                nc.tensor.matmul(
                    pv[:, i:i + 1, (D + 1) * j:(D + 1) * j + D + 1],
                    lhsT=eta[:, 128 * g:128 * (g + 1)],
                    rhs=nma, start=True, stop=True,
                )

            # rc = c/s for all 4 groups in one reciprocal
            rc4 = small_p.tile([128, 4], F32, name="rc4")
            nc.vector.reciprocal(
                rc4.rearrange("q (i j w) -> q i j w", j=2, w=1),
                pv[:, :, :2 * (D + 1)].rearrange(
                    "q i (j y) -> q i j y", y=D + 1)[:, :, :, D:D + 1],
            )
            # out_g = V_g * (c/s) + c*x_g
            # (block j of this tail covers rows n = 8q + 4h + j)
            for j in range(4):
                i, jj = divmod(j, 2)
                o_lo = (4 * h + j) * 128
                nc.vector.scalar_tensor_tensor(
                    out=o2[:, o_lo:o_lo + 128],
                    in0=pv[:, i:i + 1, (D + 1) * jj:(D + 1) * jj + D],
                    scalar=rc4[:, j:j + 1],
                    in1=xc[:, o_lo:o_lo + 128],
                    op0=OP.mult,
                    op1=OP.add,
                )

        def dma_out(p, eng=None, split=False):
            n0 = p * 2 * SB
            o2 = o2s.pop(p)
            src = o2.rearrange("q (g d) -> q g d", d=D)
            dst = out_d[n0:n0 + 2 * SB, :].rearrange("(q g) d -> q g d", g=8)
            if split:  # last pair: partition halves on separate queues
                nc.sync.dma_start(dst[:64], src[:64])
                nc.gpsimd.dma_start(dst[64:], src[64:])
            else:
                eng.dma_start(dst, src)

        # consts go on the Scalar queue (idle at start) so the Sync queue
        # issues the first xt load immediately.
        ms = consts.tile([D, K], BF16, name="ms")
        nc.scalar.dma_start(ms, ms_d)
        lw = consts.tile([K, 1], F32, name="lw")
        nc.scalar.dma_start(lw, lw_d)
        nma = consts.tile([K, D + 1], BF16, name="nma")
        nc.scalar.dma_start(nma, nma_d)

        # all input loads issued up front: inputs must never queue behind
        # output stores on the same DMA queue
        for p in range(NP):
            dma_in(p, split=(p == 0))

        for p in range(NP):
            head(2 * p)
            head(2 * p + 1)
            if p:
                tail(2 * p - 1)
                dma_out(p - 1, eng=nc.sync if p % 2 else nc.gpsimd)
            tail(2 * p)
            if p == NP - 1:
                tail(2 * p + 1)
                dma_out(p, split=True)

    nc.compile()
    return nc


def _host_constants(means, weights, alphas_cumprod, t):
    acp = float(np.asarray(alphas_cumprod, dtype=np.float64)[int(t)])
    sigma2 = 1.0 - acp
    c = 1.0 / np.sqrt(sigma2)
    mprime = np.sqrt(acp) * np.asarray(means, dtype=np.float64)      # [K, D]

    ms = (mprime / sigma2).T.astype(np.float32)                      # [D, K]
    ms = ms.astype(ml_dtypes.bfloat16)

    # Scales folded into constants: E' = E/c (via -ln c in the bias) and
    # nma = [-c*m' | 1], so the ones column accumulates s/c and
    # out = (E'@nma) * (c/s) + c*x = -(E@m')*c/s + c*x directly.
    logw = np.log(np.asarray(weights, dtype=np.float64))
    lw = (logw - 0.5 * np.sum(mprime * mprime, axis=1) / sigma2 - np.log(c))
    lw = lw.astype(np.float32).reshape(K, 1).copy()

    nma = np.zeros((K, D + 1), dtype=np.float32)
    nma[:, :D] = (-c * mprime).astype(np.float32)
    nma[:, D] = 1.0
    nma = nma.astype(ml_dtypes.bfloat16)

    return float(c), ms, lw, nma


def _host_split_x(x, c):
    # Transposed bf16 copy, columns permuted so col (1024P + 128g + q)
    # holds row (1024P + 8q + g): mm2 stationary slices stay contiguous
    # and each xc/out partition holds 8 consecutive rows (2KB DMA runs).
    v = x.reshape(-1, 128, 8, D)                        # [P, q, g, d]
    xt = np.ascontiguousarray(
        v.transpose(3, 0, 2, 1).reshape(D, -1).astype(ml_dtypes.bfloat16)
    )
    xc = (np.float32(c) * x).astype(np.float16)
    return xt, xc


def _build(inputs):
    x = np.ascontiguousarray(np.asarray(inputs["x"], dtype=np.float32))
    assert x.shape == (N, D), x.shape
    c, ms, lw, nma = _host_constants(
        inputs["means"], inputs["weights"], inputs["alphas_cumprod"], inputs["t"]
    )

    nc = build_program(c)
    in_maps = []
    for i in range(N_CORES):
        sl = slice(i * N_PER, (i + 1) * N_PER)
        xt, xc = _host_split_x(x[sl], c)
        in_maps.append({
            "xt": xt, "xc": xc,
            "ms": ms, "lw": lw, "nma": nma,
        })
    return nc, in_maps


def kernel(x, means, weights, alphas_cumprod, t):
    nc, in_maps = _build({
        "x": x, "means": means, "weights": weights,
        "alphas_cumprod": alphas_cumprod, "t": t,
    })
    res = run_bass_kernel_spmd(nc, in_maps, list(range(N_CORES)))
    out = np.concatenate([res.results[i]["out"] for i in range(N_CORES)], axis=0)
    return out.astype(np.float32, copy=False)


if __name__ == "__main__":
    rng = np.random.default_rng(0)
    x = rng.standard_normal((N, D), dtype=np.float32)
    means = 2.0 * rng.standard_normal((K, D)).astype(np.float32)
    w = rng.uniform(0.1, 1.0, K).astype(np.float32)
    weights = w / w.sum()
    betas = np.linspace(1e-4, 0.02, 1000, dtype=np.float32)
    acp = np.cumprod(1.0 - betas).astype(np.float32)
    out = kernel(x, means, weights, acp, 500)
    print("out", out.shape, out.dtype, out[:2, :4])


# revision 26
# speedup vs baseline: 1.0328x; 1.0092x over previous
"""Trainium2 Bass kernel for EpsilonNetGM (forward-diffused GMM score network).

Math (per row x of shape [D]):
    m'_k    = sqrt(acp) * means_k
    logit_k = (x . m'_k)/sigma2 + [log w_k - 0.5*||m'_k||^2/sigma2]
    resp    = softmax_k(logit)
    out     = c * (x - resp @ m'),   c = 1/sqrt(sigma2),  sigma2 = 1 - acp

Data-parallel over 8 NeuronCores: x/out sharded on the batch axis.

v3 — single-precision bf16 pipeline (tolerance is 2e-2; sim rel err 2.3e-3):
 - Host uploads x TWICE: transposed bf16 (for mm1's moving operand — no
   DMA/PE transposes on device) and c*x in f16 natural layout (for the
   final add). The transposed copy's columns are permuted so that
   n = 4p + g within each 512-row superblock: mm2 stationary slices stay
   contiguous AND the xc/out DMAs get 1KB-contiguous 4-row runs.
 - Per 512-row superblock: ONE 512-col mm1 (stationary ms [128,25]),
   one exp ACT (bias = logw_adj per-partition), FOUR 129-col mm2s
   (stationary = E^T 128-col slice, moving = [-m' | 1] so the softmax
   denominator lands in an extra PSUM column), four STTs
   out = V*(c/s) + c*x, f16 output store.
 - mm1 of superblock s+1 is issued before mm2 of superblock s so the PE
   never stalls on the exp latency.
"""

import os
import sys

for _p in ("/opt/trn_rl_repo", "/root/.axon_site/_ro/trn_rl_repo"):
    if os.path.isdir(_p) and _p not in sys.path:
        sys.path.insert(0, _p)

import numpy as np
import ml_dtypes
from contextlib import ExitStack

import concourse.bass as bass
import concourse.bacc as bacc
import concourse.tile as tile
from concourse import mybir
from concourse.bass_utils import run_bass_kernel_spmd

N_CORES = 8
N, K, D = 32768, 25, 128
N_PER = N // N_CORES          # 4096 rows per core
SB = 512                      # rows per super-block
NSB = N_PER // SB             # 8 super-blocks per core

F32 = mybir.dt.float32
F16 = mybir.dt.float16
BF16 = mybir.dt.bfloat16
AF = mybir.ActivationFunctionType
OP = mybir.AluOpType


def build_program(c_scale: float):
    nc = bacc.Bacc("TRN2", debug=False)

    xt_d = nc.dram_tensor("xt", [D, N_PER], BF16, kind="ExternalInput").ap()
    xc_d = nc.dram_tensor("xc", [N_PER, D], F16, kind="ExternalInput").ap()
    ms_d = nc.dram_tensor("ms", [D, K], BF16, kind="ExternalInput").ap()
    lw_d = nc.dram_tensor("lw", [K, 1], F32, kind="ExternalInput").ap()
    nma_d = nc.dram_tensor("nma", [K, D + 1], BF16, kind="ExternalInput").ap()
    out_d = nc.dram_tensor("out", [N_PER, D], F16, kind="ExternalOutput").ap()

    inv_c = float(1.0 / c_scale)

    NP = NSB // 2  # pairs of super-blocks (DMA granularity)

    with tile.TileContext(nc) as tc, ExitStack() as ctx:
        consts = ctx.enter_context(tc.tile_pool(name="consts", bufs=1))
        xt_p = ctx.enter_context(tc.tile_pool(name="xt", bufs=4))
        xc_p = ctx.enter_context(tc.tile_pool(name="xc", bufs=4))
        eta_p = ctx.enter_context(tc.tile_pool(name="eta", bufs=4))
        small_p = ctx.enter_context(tc.tile_pool(name="small", bufs=4))
        out_p = ctx.enter_context(tc.tile_pool(name="outp", bufs=3))
        ps_st = ctx.enter_context(tc.tile_pool(name="ps_st", bufs=2, space="PSUM"))
        ps_v = ctx.enter_context(tc.tile_pool(name="ps_v", bufs=3, space="PSUM"))

        etas, xts, xcs, o2s = {}, {}, {}, {}

        def dma_in_xt(p):
            n0 = p * 2 * SB
            # x^T slice (column-permuted: col 128g+q holds row 8q+g)
            xt = xt_p.tile([128, 2 * SB], BF16, name="xt")
            if p == 0:  # half-loads so mm1(0) starts sooner
                nc.sync.dma_start(xt[:, :SB], xt_d[:, n0:n0 + SB])
                nc.sync.dma_start(xt[:, SB:], xt_d[:, n0 + SB:n0 + 2 * SB])
            else:
                # xt is the critical input stream: pairs 1 on Sync,
                # 2 and 3 on Scalar, so nothing queues ahead of it
                eng = nc.sync if p == 1 else nc.scalar
                eng.dma_start(xt, xt_d[:, n0:n0 + 2 * SB])
            xts[p] = xt

        def dma_in_xc(p):
            n0 = p * 2 * SB
            # c*x in f16, layout [q, (g d)] <- row n0 + 8q + g: each
            # partition holds 8 consecutive rows = one 2KB contiguous run
            xc = xc_p.tile([128, 2 * SB], F16, name="xc")
            nc.gpsimd.dma_start(
                xc.rearrange("q (g d) -> q g d", d=D),
                xc_d[n0:n0 + 2 * SB, :].rearrange("(q g) d -> q g d", g=8),
            )
            xcs[p] = xc

        def head(s):
            # S^T[k, j] = x_j . m'_k / sigma2 ; E^T = exp(S^T + logw_adj)
            p, h = divmod(s, 2)
            xt = xts[p]
            pst = ps_st.tile([K, SB], F32, name="pst")
            nc.tensor.matmul(pst, lhsT=ms, rhs=xt[:, SB * h:SB * (h + 1)],
                             start=True, stop=True)
            eta = eta_p.tile([K, SB], BF16, name="eta")
            nc.scalar.activation(eta, pst, AF.Exp, bias=lw[:, 0:1], scale=1.0)
            etas[s] = eta

        def tail(s):
            p, h = divmod(s, 2)
            eta = etas.pop(s)
            xc = xcs[p]
            if h == 0:
                o2s[p] = out_p.tile([128, 2 * SB], F16, name="o2")
            o2 = o2s[p]
            # V_g = E_g @ [-c*m' | 1]; col 128 of each 129-group = s/c.
            # One 2-bank PSUM tile [q, i, 512]: group g at (i=g//2, 129*(g%2)).
            pv = ps_v.tile([128, 2, SB], F32, name="pv")
            for g in range(4):
                i, j = divmod(g, 2)
                nc.tensor.matmul(
                    pv[:, i:i + 1, (D + 1) * j:(D + 1) * j + D + 1],
                    lhsT=eta[:, 128 * g:128 * (g + 1)],
                    rhs=nma, start=True, stop=True,
                )

            # rc = c/s for all 4 groups in one reciprocal
            rc4 = small_p.tile([128, 4], F32, name="rc4")
            nc.vector.reciprocal(
                rc4.rearrange("q (i j w) -> q i j w", j=2, w=1),
                pv[:, :, :2 * (D + 1)].rearrange(
                    "q i (j y) -> q i j y", y=D + 1)[:, :, :, D:D + 1],
            )
            # out_g = V_g * (c/s) + c*x_g
            # (block j of this tail covers rows n = 8q + 4h + j)
            for j in range(4):
                i, jj = divmod(j, 2)
                o_lo = (4 * h + j) * 128
                nc.vector.scalar_tensor_tensor(
                    out=o2[:, o_lo:o_lo + 128],
                    in0=pv[:, i:i + 1, (D + 1) * jj:(D + 1) * jj + D],
                    scalar=rc4[:, j:j + 1],
                    in1=xc[:, o_lo:o_lo + 128],
                    op0=OP.mult,
                    op1=OP.add,
                )

        def dma_out(p, eng=None, split=False):
            n0 = p * 2 * SB
            o2 = o2s.pop(p)
            src = o2.rearrange("q (g d) -> q g d", d=D)
            dst = out_d[n0:n0 + 2 * SB, :].rearrange("(q g) d -> q g d", g=8)
            if split:  # last pair: partition halves on separate queues
                nc.sync.dma_start(dst[:64], src[:64])
                nc.gpsimd.dma_start(dst[64:], src[64:])
            else:
                eng.dma_start(dst, src)

        # consts + xc go on GpSimd; Sync/Scalar carry only the xt stream
        # (the critical input) so nothing delays it.
        ms = consts.tile([D, K], BF16, name="ms")
        nc.gpsimd.dma_start(ms, ms_d)
        lw = consts.tile([K, 1], F32, name="lw")
        nc.gpsimd.dma_start(lw, lw_d)
        nma = consts.tile([K, D + 1], BF16, name="nma")
        nc.gpsimd.dma_start(nma, nma_d)

        # all input loads issued up front: inputs must never queue behind
        # output stores on the same DMA queue
        for p in range(NP):
            dma_in_xt(p)
        for p in range(NP):
            dma_in_xc(p)

        for p in range(NP):
            head(2 * p)
            head(2 * p + 1)
            if p:
                tail(2 * p - 1)
                dma_out(p - 1, eng=nc.sync if p % 2 else nc.gpsimd)
            tail(2 * p)
            if p == NP - 1:
                tail(2 * p + 1)
                dma_out(p, split=True)

    nc.compile()
    return nc


def _host_constants(means, weights, alphas_cumprod, t):
    acp = float(np.asarray(alphas_cumprod, dtype=np.float64)[int(t)])
    sigma2 = 1.0 - acp
    c = 1.0 / np.sqrt(sigma2)
    mprime = np.sqrt(acp) * np.asarray(means, dtype=np.float64)      # [K, D]

    ms = (mprime / sigma2).T.astype(np.float32)                      # [D, K]
    ms = ms.astype(ml_dtypes.bfloat16)

    # Scales folded into constants: E' = E/c (via -ln c in the bias) and
    # nma = [-c*m' | 1], so the ones column accumulates s/c and
    # out = (E'@nma) * (c/s) + c*x = -(E@m')*c/s + c*x directly.
    logw = np.log(np.asarray(weights, dtype=np.float64))
    lw = (logw - 0.5 * np.sum(mprime * mprime, axis=1) / sigma2 - np.log(c))
    lw = lw.astype(np.float32).reshape(K, 1).copy()

    nma = np.zeros((K, D + 1), dtype=np.float32)
    nma[:, :D] = (-c * mprime).astype(np.float32)
    nma[:, D] = 1.0
    nma = nma.astype(ml_dtypes.bfloat16)

    return float(c), ms, lw, nma


def _host_split_x(x, c):
    # Transposed bf16 copy, columns permuted so col (1024P + 128g + q)
    # holds row (1024P + 8q + g): mm2 stationary slices stay contiguous
    # and each xc/out partition holds 8 consecutive rows (2KB DMA runs).
    v = x.reshape(-1, 128, 8, D)                        # [P, q, g, d]
    xt = np.ascontiguousarray(
        v.transpose(3, 0, 2, 1).reshape(D, -1).astype(ml_dtypes.bfloat16)
    )
    xc = (np.float32(c) * x).astype(np.float16)
    return xt, xc


def _build(inputs):
    x = np.ascontiguousarray(np.asarray(inputs["x"], dtype=np.float32))
    assert x.shape == (N, D), x.shape
    c, ms, lw, nma = _host_constants(
        inputs["means"], inputs["weights"], inputs["alphas_cumprod"], inputs["t"]
    )

    nc = build_program(c)
    in_maps = []
    for i in range(N_CORES):
        sl = slice(i * N_PER, (i + 1) * N_PER)
        xt, xc = _host_split_x(x[sl], c)
        in_maps.append({
            "xt": xt, "xc": xc,
            "ms": ms, "lw": lw, "nma": nma,
        })
    return nc, in_maps


def kernel(x, means, weights, alphas_cumprod, t):
    nc, in_maps = _build({
        "x": x, "means": means, "weights": weights,
        "alphas_cumprod": alphas_cumprod, "t": t,
    })
    res = run_bass_kernel_spmd(nc, in_maps, list(range(N_CORES)))
    out = np.concatenate([res.results[i]["out"] for i in range(N_CORES)], axis=0)
    return out.astype(np.float32, copy=False)


if __name__ == "__main__":
    rng = np.random.default_rng(0)
    x = rng.standard_normal((N, D), dtype=np.float32)
    means = 2.0 * rng.standard_normal((K, D)).astype(np.float32)
    w = rng.uniform(0.1, 1.0, K).astype(np.float32)
    weights = w / w.sum()
    betas = np.linspace(1e-4, 0.02, 1000, dtype=np.float32)
    acp = np.cumprod(1.0 - betas).astype(np.float32)
    out = kernel(x, means, weights, acp, 500)
    print("out", out.shape, out.dtype, out[:2, :4])


# revision 31
# speedup vs baseline: 1.0448x; 1.0116x over previous
"""Trainium2 Bass kernel for EpsilonNetGM (forward-diffused GMM score network).

Math (per row x of shape [D]):
    m'_k    = sqrt(acp) * means_k
    logit_k = (x . m'_k)/sigma2 + [log w_k - 0.5*||m'_k||^2/sigma2]
    resp    = softmax_k(logit)
    out     = c * (x - resp @ m'),   c = 1/sqrt(sigma2),  sigma2 = 1 - acp

Data-parallel over 8 NeuronCores: x/out sharded on the batch axis.

v3 — single-precision bf16 pipeline (tolerance is 2e-2; sim rel err 2.3e-3):
 - Host uploads x TWICE: transposed bf16 (for mm1's moving operand — no
   DMA/PE transposes on device) and c*x in f16 natural layout (for the
   final add). The transposed copy's columns are permuted so that
   n = 4p + g within each 512-row superblock: mm2 stationary slices stay
   contiguous AND the xc/out DMAs get 1KB-contiguous 4-row runs.
 - Per 512-row superblock: ONE 512-col mm1 (stationary ms [128,25]),
   one exp ACT (bias = logw_adj per-partition), FOUR 129-col mm2s
   (stationary = E^T 128-col slice, moving = [-m' | 1] so the softmax
   denominator lands in an extra PSUM column), four STTs
   out = V*(c/s) + c*x, f16 output store.
 - mm1 of superblock s+1 is issued before mm2 of superblock s so the PE
   never stalls on the exp latency.
"""

import os
import sys

for _p in ("/opt/trn_rl_repo", "/root/.axon_site/_ro/trn_rl_repo"):
    if os.path.isdir(_p) and _p not in sys.path:
        sys.path.insert(0, _p)

import numpy as np
import ml_dtypes
from contextlib import ExitStack

import concourse.bass as bass
import concourse.bacc as bacc
import concourse.tile as tile
from concourse import mybir
from concourse.bass_utils import run_bass_kernel_spmd

N_CORES = 8
N, K, D = 32768, 25, 128
N_PER = N // N_CORES          # 4096 rows per core
SB = 512                      # rows per super-block
NSB = N_PER // SB             # 8 super-blocks per core

F32 = mybir.dt.float32
F16 = mybir.dt.float16
BF16 = mybir.dt.bfloat16
AF = mybir.ActivationFunctionType
OP = mybir.AluOpType


def build_program(c_scale: float):
    nc = bacc.Bacc("TRN2", debug=False)

    # xt carries ms in its first K columns: the ms load rides the first
    # xt DMA's descriptors instead of paying its own 128-run ring cost.
    xt_d = nc.dram_tensor("xt", [D, K + N_PER], BF16, kind="ExternalInput").ap()
    xc_d = nc.dram_tensor("xc", [N_PER, D], F16, kind="ExternalInput").ap()
    lw_d = nc.dram_tensor("lw", [K, 1], F32, kind="ExternalInput").ap()
    nma_d = nc.dram_tensor("nma", [K, D + 1], BF16, kind="ExternalInput").ap()
    out_d = nc.dram_tensor("out", [N_PER, D], F16, kind="ExternalOutput").ap()

    inv_c = float(1.0 / c_scale)

    NP = NSB // 2  # pairs of super-blocks (DMA granularity)

    with tile.TileContext(nc) as tc, ExitStack() as ctx:
        consts = ctx.enter_context(tc.tile_pool(name="consts", bufs=1))
        xt_p = ctx.enter_context(tc.tile_pool(name="xt", bufs=4))
        xc_p = ctx.enter_context(tc.tile_pool(name="xc", bufs=4))
        eta_p = ctx.enter_context(tc.tile_pool(name="eta", bufs=4))
        small_p = ctx.enter_context(tc.tile_pool(name="small", bufs=4))
        out_p = ctx.enter_context(tc.tile_pool(name="outp", bufs=3))
        ps_st = ctx.enter_context(tc.tile_pool(name="ps_st", bufs=2, space="PSUM"))
        ps_v = ctx.enter_context(tc.tile_pool(name="ps_v", bufs=3, space="PSUM"))

        etas, xts, xcs, o2s = {}, {}, {}, {}

        def dma_in_xt(p):
            n0 = K + p * 2 * SB
            # x^T slice (column-permuted: col 128g+q holds row 8q+g)
            if p == 0:  # ms rides along; half-loads so mm1(0) starts sooner
                xt = xt_p.tile([128, K + 2 * SB], BF16, name="xt0")
                nc.sync.dma_start(xt[:, :K + SB], xt_d[:, :K + SB])
                nc.sync.dma_start(xt[:, K + SB:], xt_d[:, K + SB:K + 2 * SB])
                xts[p] = xt[:, K:]
                return xt[:, :K]
            # xt is the critical input stream: pair 1 on Sync,
            # 2 and 3 on Scalar, so nothing queues ahead of it
            xt = xt_p.tile([128, 2 * SB], BF16, name="xt")
            eng = nc.sync if p == 1 else nc.scalar
            eng.dma_start(xt, xt_d[:, n0:n0 + 2 * SB])
            xts[p] = xt

        def dma_in_xc(p):
            n0 = p * 2 * SB
            # c*x in f16, layout [q, (g d)] <- row n0 + 8q + g: each
            # partition holds 8 consecutive rows = one 2KB contiguous run
            xc = xc_p.tile([128, 2 * SB], F16, name="xc")
            nc.gpsimd.dma_start(
                xc.rearrange("q (g d) -> q g d", d=D),
                xc_d[n0:n0 + 2 * SB, :].rearrange("(q g) d -> q g d", g=8),
            )
            xcs[p] = xc

        def head(s):
            # S^T[k, j] = x_j . m'_k / sigma2 ; E^T = exp(S^T + logw_adj)
            p, h = divmod(s, 2)
            xt = xts[p]
            pst = ps_st.tile([K, SB], F32, name="pst")
            nc.tensor.matmul(pst, lhsT=ms, rhs=xt[:, SB * h:SB * (h + 1)],
                             start=True, stop=True)
            eta = eta_p.tile([K, SB], BF16, name="eta")
            nc.scalar.activation(eta, pst, AF.Exp, bias=lw[:, 0:1], scale=1.0)
            etas[s] = eta

        def tail(s):
            p, h = divmod(s, 2)
            eta = etas.pop(s)
            xc = xcs[p]
            if h == 0:
                o2s[p] = out_p.tile([128, 2 * SB], F16, name="o2")
            o2 = o2s[p]
            # V_g = E_g @ [-c*m' | 1]; col 128 of each 129-group = s/c.
            # One 2-bank PSUM tile [q, i, 512]: group g at (i=g//2, 129*(g%2)).
            pv = ps_v.tile([128, 2, SB], F32, name="pv")
            for g in range(4):
                i, j = divmod(g, 2)
                nc.tensor.matmul(
                    pv[:, i:i + 1, (D + 1) * j:(D + 1) * j + D + 1],
                    lhsT=eta[:, 128 * g:128 * (g + 1)],
                    rhs=nma, start=True, stop=True,
                )

            # rc = c/s for all 4 groups in one reciprocal
            rc4 = small_p.tile([128, 4], F32, name="rc4")
            nc.vector.reciprocal(
                rc4.rearrange("q (i j w) -> q i j w", j=2, w=1),
                pv[:, :, :2 * (D + 1)].rearrange(
                    "q i (j y) -> q i j y", y=D + 1)[:, :, :, D:D + 1],
            )
            # out_g = V_g * (c/s) + c*x_g
            # (block j of this tail covers rows n = 8q + 4h + j)
            for j in range(4):
                i, jj = divmod(j, 2)
                o_lo = (4 * h + j) * 128
                nc.vector.scalar_tensor_tensor(
                    out=o2[:, o_lo:o_lo + 128],
                    in0=pv[:, i:i + 1, (D + 1) * jj:(D + 1) * jj + D],
                    scalar=rc4[:, j:j + 1],
                    in1=xc[:, o_lo:o_lo + 128],
                    op0=OP.mult,
                    op1=OP.add,
                )

        def dma_out(p, eng=None, split=False):
            n0 = p * 2 * SB
            o2 = o2s.pop(p)
            src = o2.rearrange("q (g d) -> q g d", d=D)
            dst = out_d[n0:n0 + 2 * SB, :].rearrange("(q g) d -> q g d", g=8)
            if split:  # last pair: partition halves on separate queues
                nc.sync.dma_start(dst[:64], src[:64])
                nc.gpsimd.dma_start(dst[64:], src[64:])
            else:
                eng.dma_start(dst, src)

        # lw/nma go first on the GpSimd ring (tiny, needed early);
        # Sync/Scalar carry only the xt stream so nothing delays it.
        lw = consts.tile([K, 1], F32, name="lw")
        nc.gpsimd.dma_start(lw, lw_d)
        nma = consts.tile([K, D + 1], BF16, name="nma")
        nc.gpsimd.dma_start(nma, nma_d)

        # all input loads issued up front: inputs must never queue behind
        # output stores on the same DMA queue
        ms = None
        for p in range(NP):
            r = dma_in_xt(p)
            if p == 0:
                ms = r
        for p in range(NP):
            dma_in_xc(p)

        for p in range(NP):
            head(2 * p)
            head(2 * p + 1)
            if p:
                tail(2 * p - 1)
                dma_out(p - 1, eng=nc.sync if p % 2 else nc.gpsimd)
            tail(2 * p)
            if p == NP - 1:
                tail(2 * p + 1)
                dma_out(p, split=True)

    nc.compile()
    return nc


def _host_constants(means, weights, alphas_cumprod, t):
    acp = float(np.asarray(alphas_cumprod, dtype=np.float64)[int(t)])
    sigma2 = 1.0 - acp
    c = 1.0 / np.sqrt(sigma2)
    mprime = np.sqrt(acp) * np.asarray(means, dtype=np.float64)      # [K, D]

    ms = (mprime / sigma2).T.astype(np.float32)                      # [D, K]
    ms = ms.astype(ml_dtypes.bfloat16)

    # Scales folded into constants: E' = E/c (via -ln c in the bias) and
    # nma = [-c*m' | 1], so the ones column accumulates s/c and
    # out = (E'@nma) * (c/s) + c*x = -(E@m')*c/s + c*x directly.
    logw = np.log(np.asarray(weights, dtype=np.float64))
    lw = (logw - 0.5 * np.sum(mprime * mprime, axis=1) / sigma2 - np.log(c))
    lw = lw.astype(np.float32).reshape(K, 1).copy()

    nma = np.zeros((K, D + 1), dtype=np.float32)
    nma[:, :D] = (-c * mprime).astype(np.float32)
    nma[:, D] = 1.0
    nma = nma.astype(ml_dtypes.bfloat16)

    return float(c), ms, lw, nma


def _host_split_x(x, c):
    # Transposed bf16 copy, columns permuted so col (1024P + 128g + q)
    # holds row (1024P + 8q + g): mm2 stationary slices stay contiguous
    # and each xc/out partition holds 8 consecutive rows (2KB DMA runs).
    v = x.reshape(-1, 128, 8, D)                        # [P, q, g, d]
    xt = v.transpose(3, 0, 2, 1).reshape(D, -1).astype(ml_dtypes.bfloat16)
    xc = (np.float32(c) * x).astype(np.float16)
    return xt, xc


def _build(inputs):
    x = np.ascontiguousarray(np.asarray(inputs["x"], dtype=np.float32))
    assert x.shape == (N, D), x.shape
    c, ms, lw, nma = _host_constants(
        inputs["means"], inputs["weights"], inputs["alphas_cumprod"], inputs["t"]
    )

    nc = build_program(c)
    in_maps = []
    for i in range(N_CORES):
        sl = slice(i * N_PER, (i + 1) * N_PER)
        xt, xc = _host_split_x(x[sl], c)
        xtm = np.ascontiguousarray(np.concatenate([ms, xt], axis=1))
        in_maps.append({
            "xt": xtm, "xc": xc,
            "lw": lw, "nma": nma,
        })
    return nc, in_maps


def kernel(x, means, weights, alphas_cumprod, t):
    nc, in_maps = _build({
        "x": x, "means": means, "weights": weights,
        "alphas_cumprod": alphas_cumprod, "t": t,
    })
    res = run_bass_kernel_spmd(nc, in_maps, list(range(N_CORES)))
    out = np.concatenate([res.results[i]["out"] for i in range(N_CORES)], axis=0)
    return out.astype(np.float32, copy=False)


if __name__ == "__main__":
    rng = np.random.default_rng(0)
    x = rng.standard_normal((N, D), dtype=np.float32)
    means = 2.0 * rng.standard_normal((K, D)).astype(np.float32)
    w = rng.uniform(0.1, 1.0, K).astype(np.float32)
    weights = w / w.sum()
    betas = np.linspace(1e-4, 0.02, 1000, dtype=np.float32)
    acp = np.cumprod(1.0 - betas).astype(np.float32)
    out = kernel(x, means, weights, acp, 500)
    print("out", out.shape, out.dtype, out[:2, :4])
